# revision 3
# baseline (speedup 1.0000x reference)
"""Trainium2 Bass kernel for nn_GAT_14946486190732.

Math: the reference builds a chain graph where edge i connects src node i to
dst node i (u = v = arange(E)), so every dst segment in the edge softmax has
exactly one edge: segment_max == the score itself, exp(0) == 1, denom == 1,
alpha == 1 exactly. The whole attention branch is a no-op, and

    out[b, 0,  :] = loc[b, 0, :]
    out[b, i,  :] = loc[b, i-1, :] @ A^T + loc[b, i, :] @ B^T + c   (i >= 1)

with A = mean_h W_src.reshape(H,F,F), B = mean_h W_res.reshape(H,F,F),
c = mean_h bias.reshape(H,F)  (head-mean folded into the weights).

Device strategy (8 cores, data-parallel over batch, 4 samples/core):
  - host pre-transposes loc to (B, F, L) so features sit on SBUF partitions
  - per 512-row chunk: two PSUM-accumulated matmuls with the weights as the
    stationary operand and shifted xT windows as the 512-wide moving operand
    (float32r -> full PE rate), bias-add fused into the PSUM->SBUF copy on
    the scalar engine, 4 PE transposes back to natural layout, one DVE copy,
    one batched 256KB store DMA.
"""

import numpy as np

from concourse import bass, bacc, tile, mybir
from concourse.bass_utils import run_bass_kernel_spmd

F32 = mybir.dt.float32
F32R = mybir.dt.float32r

N_CORES = 8
B_FULL, L, F = 32, 4096, 128
B_SH = B_FULL // N_CORES  # samples per core
N_CHUNKS = 8  # 512-row chunks per sample
CW = 512  # chunk width

# Matmul input dtype: float32r runs the PE at full rate for moving dims >= 256
# (plain float32 is 4 cycles/row). Flip to False if precision ever demands it.
USE_F32R = True
# Store the result feature-major and un-transpose on the host: halves the PE
# work (no on-device output transposes) and removes the DVE stage entirely.
TRANSPOSED_OUT = True


def _build_program(
    use_f32r=None,
    repeat=1,
    transposed_out=None,
    copy_engine="act",
    psum_bufs=6,
    ot_bufs=3,
    xt_bufs=3,
    store_hwdge=False,
    store_every=1,
    load_hwdge=False,
    preload=False,
    bigstore=True,
):
    if use_f32r is None:
        use_f32r = USE_F32R
    if transposed_out is None:
        transposed_out = TRANSPOSED_OUT
    nc = bacc.Bacc(
        "TRN2",
        target_bir_lowering=False,
        num_devices=N_CORES,
        num_swdge_queues=4,
    )

    mm_dt = F32R if use_f32r else F32

    xt = nc.declare_dram_parameter("xt", [B_SH, F, L], mm_dt, isOutput=False)
    wa = nc.declare_dram_parameter("wa", [F, F], mm_dt, isOutput=False)
    wb = nc.declare_dram_parameter("wb", [F, F], mm_dt, isOutput=False)
    cb = nc.declare_dram_parameter("cb", [F, 1], F32, isOutput=False)
    ident = nc.declare_dram_parameter("ident", [F, F], F32, isOutput=False)
    oshape = [B_SH, F, L] if transposed_out else [B_SH, L, F]
    out = nc.declare_dram_parameter("out", oshape, F32, isOutput=True)

    with tile.TileContext(nc) as tc:
        with (
            tc.tile_pool(name="consts", bufs=1) as consts,
            tc.tile_pool(name="xtp", bufs=xt_bufs) as xtp,
            tc.tile_pool(name="outT", bufs=ot_bufs) as outTp,
            tc.tile_pool(name="obig", bufs=2) as obigp,
            tc.tile_pool(name="onat", bufs=3) as onatp,
            tc.tile_pool(name="pmm", bufs=psum_bufs, space="PSUM") as pmmp,
            tc.tile_pool(name="ptr", bufs=2, space="PSUM") as ptrp,
        ):
            wa_sb = consts.tile([F, F], mm_dt)
            wb_sb = consts.tile([F, F], mm_dt)
            cb_sb = consts.tile([F, 1], F32)
            id_sb = consts.tile([F, F], F32)
            nc.gpsimd.dma_start(out=wa_sb[:], in_=wa[:])
            nc.gpsimd.dma_start(out=wb_sb[:], in_=wb[:])
            nc.gpsimd.dma_start(out=cb_sb[:], in_=cb[:])
            nc.gpsimd.dma_start(out=id_sb[:], in_=ident[:])

            for _rep in range(repeat):
              ld_eng = nc.sync if load_hwdge else nc.gpsimd
              if preload:
                  xts = []
                  for b in range(B_SH):
                      xt_sb = xtp.tile([F, L], mm_dt)
                      ld_eng.dma_start(out=xt_sb[:], in_=xt[b])
                      xts.append(xt_sb)
              for b in range(B_SH):
                if preload:
                    xt_sb = xts[b]
                else:
                    xt_sb = xtp.tile([F, L], mm_dt)
                    ld_eng.dma_start(out=xt_sb[:], in_=xt[b])
                if bigstore:
                    obig = obigp.tile([F, L], F32)

                for k in range(N_CHUNKS):
                    # last chunk starts one row early so all chunks are 512
                    # wide; res row 3583 is computed (identically) twice.
                    r0 = 512 * k if k < N_CHUNKS - 1 else L - 1 - CW
                    pm = pmmp.tile([F, CW], F32)
                    # pm[o, n] = sum_e A[o,e] * x[r0+n, e]  (+ B against x+1)
                    nc.tensor.matmul(
                        pm[:],
                        lhsT=wa_sb[:],
                        rhs=xt_sb[:, r0 : r0 + CW],
                        start=True,
                        stop=False,
                    )
                    nc.tensor.matmul(
                        pm[:],
                        lhsT=wb_sb[:],
                        rhs=xt_sb[:, r0 + 1 : r0 + 1 + CW],
                        start=False,
                        stop=True,
                    )
                    # PSUM -> SBUF with per-partition bias add (out = in + c)
                    if bigstore:
                        ot = obig[:, 1 + r0 : 1 + r0 + CW]
                    else:
                        ot = outTp.tile([F, CW], F32)
                    eng = copy_engine
                    if eng == "alt":
                        eng = "act" if (k % 2 == 0) else "dve"
                    if eng == "act":
                        nc.scalar.add(ot[:], pm[:], cb_sb[:])
                    elif eng == "split":
                        h = CW // 2
                        nc.scalar.add(ot[:, :h], pm[:, :h], cb_sb[:])
                        nc.vector.tensor_scalar_add(
                            ot[:, h:], pm[:, h:], cb_sb[:]
                        )
                    else:
                        nc.vector.tensor_scalar_add(ot[:], pm[:], cb_sb[:])
                    if transposed_out:
                        # store transposed; host un-transposes afterwards
                        st_eng = nc.sync if store_hwdge else nc.gpsimd
                        if bigstore:
                            if k == N_CHUNKS - 1:
                                st_eng.dma_start(
                                    out=out[b, :, 1:], in_=obig[:, 1:]
                                )
                        elif k % store_every == 0:
                            st_eng.dma_start(
                                out=out[b, :, 1 + r0 : 1 + r0 + CW], in_=ot[:]
                            )
                    else:
                        # transpose back to natural [row, feature] layout
                        pt = ptrp.tile([F, CW], F32)
                        for j in range(4):
                            nc.tensor.transpose(
                                pt[:, 128 * j : 128 * (j + 1)],
                                ot[:, 128 * j : 128 * (j + 1)],
                                id_sb[:],
                            )
                        on = onatp.tile([F, CW], F32)
                        nc.vector.tensor_copy(on[:], pt[:])
                        # store rows [1+r0, 1+r0+512) of out[b]
                        dst = out[b, 1 + r0 : 1 + r0 + CW, :].rearrange(
                            "(j p) f -> p j f", p=128
                        )
                        src = on[:].rearrange("p (j f) -> p j f", j=4)
                        nc.gpsimd.dma_start(out=dst, in_=src)

    nc.compile()
    return nc


# test.py toggles these to capture an NTFF/perfetto profile of the run; the
# grading harness never touches them (TRACE defaults False).
TRACE = False
TRACE_CORES = None  # e.g. [0] or list(range(N_CORES))
TRACE_TMPDIR = None
LAST_RESULT = None

_NC_CACHE = {}


def _get_program(use_f32r=None, repeat=1, transposed_out=None):
    if use_f32r is None:
        use_f32r = USE_F32R
    if transposed_out is None:
        transposed_out = TRANSPOSED_OUT
    key = (use_f32r, repeat, transposed_out)
    if key not in _NC_CACHE:
        _NC_CACHE[key] = _build_program(use_f32r, repeat, transposed_out)
    return _NC_CACHE[key]


def kernel(loc, W_src, W_dst, attn_l, attn_r, W_res, bias):
    loc = np.ascontiguousarray(np.asarray(loc, dtype=np.float32))
    H = 8
    A = np.asarray(W_src, np.float32).reshape(H, F, F).mean(axis=0)
    Bm = np.asarray(W_res, np.float32).reshape(H, F, F).mean(axis=0)
    c = np.asarray(bias, np.float32).reshape(H, F).mean(axis=0)

    # feature-major inputs for the device (features on SBUF partitions)
    xt_full = np.ascontiguousarray(loc.transpose(0, 2, 1))  # (B, F, L)
    wa = np.ascontiguousarray(A.T)  # wa[e, o] = A[o, e]
    wb = np.ascontiguousarray(Bm.T)
    cbv = np.ascontiguousarray(c.reshape(F, 1))
    ident = np.eye(F, dtype=np.float32)

    in_maps = [
        {
            "xt": np.ascontiguousarray(xt_full[i * B_SH : (i + 1) * B_SH]),
            "wa": wa,
            "wb": wb,
            "cb": cbv,
            "ident": ident,
        }
        for i in range(N_CORES)
    ]

    nc = _get_program()
    kw = {}
    if TRACE:
        kw = dict(
            trace=True,
            trace_cores=TRACE_CORES if TRACE_CORES is not None else [0],
            tmpdir=TRACE_TMPDIR,
        )
    res = run_bass_kernel_spmd(nc, in_maps, list(range(N_CORES)), **kw)
    if TRACE:
        global LAST_RESULT
        LAST_RESULT = res

    out = np.empty((B_FULL, L, F), dtype=np.float32)
    if TRANSPOSED_OUT:
        for i in range(N_CORES):
            out[i * B_SH : (i + 1) * B_SH] = res.results[i]["out"].transpose(
                0, 2, 1
            )
    else:
        for i in range(N_CORES):
            out[i * B_SH : (i + 1) * B_SH] = res.results[i]["out"]
    out[:, 0, :] = loc[:, 0, :]  # origin row passthrough
    return out



# revision 4
# speedup vs baseline: 1.4340x; 1.4340x over previous
"""Trainium2 Bass kernel for nn_GAT_14946486190732.

Math: the reference builds a chain graph where edge i connects src node i to
dst node i (u = v = arange(E)), so every dst segment in the edge softmax has
exactly one edge: segment_max == the score itself, exp(0) == 1, denom == 1,
alpha == 1 exactly. The whole attention branch is a no-op, and

    out[b, 0,  :] = loc[b, 0, :]
    out[b, i,  :] = loc[b, i-1, :] @ A^T + loc[b, i, :] @ B^T + c   (i >= 1)

with A = mean_h W_src.reshape(H,F,F), B = mean_h W_res.reshape(H,F,F),
c = mean_h bias.reshape(H,F)  (head-mean folded into the weights).

Device strategy (8 cores, data-parallel over batch, 4 samples/core):
  - the kernel is HBM-bandwidth-bound, so the streamed tensors (loc in,
    result out) travel as bfloat16: host pre-transposes loc to (B, F, L)
    and casts to bf16; the device returns the result feature-major in bf16
    and the host un-transposes/upcasts. End-to-end rel err ~2.6e-3 vs the
    2e-2 gate.
  - per 512-row chunk: two PSUM-accumulated bf16 matmuls (full PE rate)
    with the weights stationary and shifted xT windows as the 512-wide
    moving operand; bias-add fused into the PSUM->SBUF downcast copy,
    alternating ACT/DVE so neither engine bottlenecks; one batched 1MB
    store DMA per sample.
"""

import numpy as np
import ml_dtypes

from concourse import bass, bacc, tile, mybir
from concourse.bass_utils import run_bass_kernel_spmd

F32 = mybir.dt.float32
BF16 = mybir.dt.bfloat16
NP_BF16 = ml_dtypes.bfloat16

N_CORES = 8
B_FULL, L, F = 32, 4096, 128
B_SH = B_FULL // N_CORES  # samples per core
N_CHUNKS = 8  # 512-col chunks per sample
CW = 512  # chunk width (one PSUM bank: 512 * 4B = 2KB/partition)


def _build_program(xt_bufs=3, obig_bufs=2, psum_bufs=6):
    nc = bacc.Bacc(
        "TRN2",
        target_bir_lowering=False,
        num_devices=N_CORES,
        num_swdge_queues=4,
    )

    xt = nc.declare_dram_parameter("xt", [B_SH, F, L], BF16, isOutput=False)
    wa = nc.declare_dram_parameter("wa", [F, F], BF16, isOutput=False)
    wb = nc.declare_dram_parameter("wb", [F, F], BF16, isOutput=False)
    cb = nc.declare_dram_parameter("cb", [F, 1], F32, isOutput=False)
    out = nc.declare_dram_parameter("out", [B_SH, F, L], BF16, isOutput=True)

    with tile.TileContext(nc) as tc:
        with (
            tc.tile_pool(name="consts", bufs=1) as consts,
            tc.tile_pool(name="xtp", bufs=xt_bufs) as xtp,
            tc.tile_pool(name="obig", bufs=obig_bufs) as obigp,
            tc.tile_pool(name="pmm", bufs=psum_bufs, space="PSUM") as pmmp,
        ):
            wa_sb = consts.tile([F, F], BF16)
            wb_sb = consts.tile([F, F], BF16)
            cb_sb = consts.tile([F, 1], F32)
            nc.gpsimd.dma_start(out=wa_sb[:], in_=wa[:])
            nc.gpsimd.dma_start(out=wb_sb[:], in_=wb[:])
            nc.gpsimd.dma_start(out=cb_sb[:], in_=cb[:])

            for b in range(B_SH):
                xt_sb = xtp.tile([F, L], BF16)
                nc.gpsimd.dma_start(out=xt_sb[:], in_=xt[b])
                obig = obigp.tile([F, L], BF16)

                for k in range(N_CHUNKS):
                    # last chunk starts one col early so all chunks are 512
                    # wide; output col 3584 is computed (identically) twice.
                    r0 = 512 * k if k < N_CHUNKS - 1 else L - 1 - CW
                    pm = pmmp.tile([F, CW], F32)
                    # pm[o, n] = sum_e A[o,e] * x[r0+n, e]  (+ B against x+1)
                    nc.tensor.matmul(
                        pm[:],
                        lhsT=wa_sb[:],
                        rhs=xt_sb[:, r0 : r0 + CW],
                        start=True,
                        stop=False,
                    )
                    nc.tensor.matmul(
                        pm[:],
                        lhsT=wb_sb[:],
                        rhs=xt_sb[:, r0 + 1 : r0 + 1 + CW],
                        start=False,
                        stop=True,
                    )
                    # PSUM -> SBUF downcast with per-partition bias add,
                    # alternating engines so copies never bottleneck.
                    ot = obig[:, 1 + r0 : 1 + r0 + CW]
                    if k % 2 == 0:
                        nc.scalar.add(ot, pm[:], cb_sb[:])
                    else:
                        nc.vector.tensor_scalar_add(ot, pm[:], cb_sb[:])

                # col 0 of obig is never written; the host overwrites output
                # row 0 with loc[:, 0] anyway, so store full width (aligned
                # 8KB/partition descriptors).
                nc.gpsimd.dma_start(out=out[b], in_=obig[:])

    nc.compile()
    return nc


# test.py toggles these to capture an NTFF/perfetto profile of the run; the
# grading harness never touches them (TRACE defaults False).
TRACE = False
TRACE_CORES = None  # e.g. [0] or list(range(N_CORES))
TRACE_TMPDIR = None
LAST_RESULT = None

_NC_CACHE = {}


def _get_program():
    if "nc" not in _NC_CACHE:
        _NC_CACHE["nc"] = _build_program()
    return _NC_CACHE["nc"]


def kernel(loc, W_src, W_dst, attn_l, attn_r, W_res, bias):
    loc = np.asarray(loc, dtype=np.float32)
    H = 8
    A = np.asarray(W_src, np.float32).reshape(H, F, F).mean(axis=0)
    Bm = np.asarray(W_res, np.float32).reshape(H, F, F).mean(axis=0)
    c = np.asarray(bias, np.float32).reshape(H, F).mean(axis=0)

    # feature-major bf16 inputs for the device (features on SBUF partitions)
    xt_full = np.ascontiguousarray(
        loc.transpose(0, 2, 1).astype(NP_BF16)
    )  # (B, F, L)
    wa = np.ascontiguousarray(A.T.astype(NP_BF16))  # wa[e, o] = A[o, e]
    wb = np.ascontiguousarray(Bm.T.astype(NP_BF16))
    cbv = np.ascontiguousarray(c.reshape(F, 1))

    in_maps = [
        {
            "xt": np.ascontiguousarray(xt_full[i * B_SH : (i + 1) * B_SH]),
            "wa": wa,
            "wb": wb,
            "cb": cbv,
        }
        for i in range(N_CORES)
    ]

    nc = _get_program()
    kw = {}
    if TRACE:
        kw = dict(
            trace=True,
            trace_cores=TRACE_CORES if TRACE_CORES is not None else [0],
            tmpdir=TRACE_TMPDIR,
        )
    res = run_bass_kernel_spmd(nc, in_maps, list(range(N_CORES)), **kw)
    if TRACE:
        global LAST_RESULT
        LAST_RESULT = res

    out = np.empty((B_FULL, L, F), dtype=np.float32)
    for i in range(N_CORES):
        out[i * B_SH : (i + 1) * B_SH] = (
            res.results[i]["out"].astype(np.float32).transpose(0, 2, 1)
        )
    out[:, 0, :] = loc[:, 0, :]  # origin row passthrough
    return out


# revision 10
# speedup vs baseline: 1.6295x; 1.1363x over previous
"""Trainium2 Bass kernel for nn_GAT_14946486190732.

Math: the reference builds a chain graph where edge i connects src node i to
dst node i (u = v = arange(E)), so every dst segment in the edge softmax has
exactly one edge: segment_max == the score itself, exp(0) == 1, denom == 1,
alpha == 1 exactly. The whole attention branch is a no-op, and

    out[b, 0,  :] = loc[b, 0, :]
    out[b, i,  :] = loc[b, i-1, :] @ A^T + loc[b, i, :] @ B^T + c   (i >= 1)

with A = mean_h W_src.reshape(H,F,F), B = mean_h W_res.reshape(H,F,F),
c = mean_h bias.reshape(H,F)  (head-mean folded into the weights).

Device strategy (8 cores, data-parallel over batch, 4 samples/core):
  - HBM-bandwidth-bound, so the streamed tensors travel as bfloat16 (host
    pre-transposes loc to (B, F, L) + casts; device returns feature-major
    bf16, host un-transposes/upcasts). Rel err ~2.6e-3 vs the 2e-2 gate.
  - loads issue from the idle sync engine (HWDGE queue) so they are never
    stuck behind store packets; stores issue from gpsimd (SWDGE queue).
  - sample 0's load is split in two so the first matmul starts as soon as
    ~0.5 MB lands; samples 1-3 load as single 1 MB DMAs (8 KB/partition
    lines, no runt packets). Stores are whole-sample for samples 0-2 and
    1024-col groups for the last sample to shorten the drain tail.
  - per 512-col chunk: two PSUM-accumulated bf16 matmuls (full PE rate,
    weights stationary); bias-add fused into the PSUM->SBUF downcast copy,
    alternating ACT/DVE; ACT's activation table is pre-warmed during the
    first load so the table load is off the critical path.
"""

import numpy as np
import ml_dtypes

from concourse import bass, bacc, tile, mybir
from concourse.bass_utils import run_bass_kernel_spmd

F32 = mybir.dt.float32
BF16 = mybir.dt.bfloat16
NP_BF16 = ml_dtypes.bfloat16

N_CORES = 8
B_FULL, L, F = 32, 4096, 128
B_SH = B_FULL // N_CORES  # samples per core
N_CHUNKS = 8  # 512-col matmul chunks per sample
CW = 512  # matmul chunk width (one PSUM bank: 512 * 4B = 2KB/partition)
S0_SPLIT = 2049  # sample-0 first piece: cols [0, 2049) (covers chunks 0-3)
S0_B0 = 2048  # sample-0 second piece starts here (1-col overlap with piece 1)


def _build_program():
    nc = bacc.Bacc(
        "TRN2",
        target_bir_lowering=False,
        num_devices=N_CORES,
        num_swdge_queues=4,
    )

    xt = nc.declare_dram_parameter("xt", [B_SH, F, L], BF16, isOutput=False)
    # wab[:, 0:128] = A^T, wab[:, 128:256] = B^T (single load DMA)
    wab = nc.declare_dram_parameter("wab", [F, 2 * F], BF16, isOutput=False)
    cb = nc.declare_dram_parameter("cb", [F, 1], F32, isOutput=False)
    out = nc.declare_dram_parameter("out", [B_SH, F, L], BF16, isOutput=True)

    with tile.TileContext(nc) as tc:
        with (
            tc.tile_pool(name="consts", bufs=1) as consts,
            tc.tile_pool(name="xtp", bufs=3) as xtp,  # whole-sample tiles
            tc.tile_pool(name="xsp", bufs=2) as xsp,  # sample-0 split tiles
            tc.tile_pool(name="obig", bufs=2) as obigp,
            tc.tile_pool(name="ostg", bufs=4) as ostgp,  # last-sample groups
            tc.tile_pool(name="pmm", bufs=6, space="PSUM") as pmmp,
        ):
            wab_sb = consts.tile([F, 2 * F], BF16)
            cb_sb = consts.tile([F, 1], F32)
            warm = consts.tile([F, 1], F32)
            nc.sync.dma_start(out=wab_sb[:], in_=wab[:])
            nc.sync.dma_start(out=cb_sb[:], in_=cb[:])
            # pull ACT's activation-table load off the critical path: runs
            # while sample 0 is still streaming in
            nc.scalar.add(warm[:], cb_sb[:], cb_sb[:])

            wa_sb = wab_sb[:, 0:F]
            wb_sb = wab_sb[:, F : 2 * F]

            # ---- loads (sync engine / HWDGE): issue everything up front ----
            # sample 0 split so matmuls start after ~0.5MB, not 1MB
            x0a = xsp.tile([F, S0_SPLIT], BF16)
            nc.sync.dma_start(out=x0a[:], in_=xt[0, :, 0:S0_SPLIT])
            x0b = xsp.tile([F, L - S0_B0], BF16)
            nc.sync.dma_start(out=x0b[:], in_=xt[0, :, S0_B0:L])
            xts = []
            for b in range(1, B_SH):
                t = xtp.tile([F, L], BF16)
                nc.sync.dma_start(out=t[:], in_=xt[b])
                xts.append(t)

            def sample_windows(b, r0, w):
                """SBUF view of xt[b, :, r0:r0+w] given the load tiling."""
                if b == 0:
                    if r0 + w <= S0_SPLIT:
                        return x0a[:, r0 : r0 + w]
                    assert r0 >= S0_B0, (r0, w)
                    return x0b[:, r0 - S0_B0 : r0 - S0_B0 + w]
                return xts[b - 1][:, r0 : r0 + w]

            # ---- compute + stores ----
            for b in range(B_SH):
                last = b == B_SH - 1
                if not last:
                    obig = obigp.tile([F, L], BF16)
                    ogs = None
                else:
                    # 1024-col store groups: dest cols 1/1025/2049/3073
                    ogs = [
                        ostgp.tile([F, 1024 if g < 3 else 1023], BF16, name=f"og{g}")
                        for g in range(4)
                    ]

                for k in range(N_CHUNKS):
                    # last chunk starts one col early so all chunks are 512
                    # wide; its first output col is dropped at the copy.
                    r0 = CW * k if k < N_CHUNKS - 1 else L - 1 - CW
                    pm = pmmp.tile([F, CW], F32)
                    # pm[o, n] = sum_e A[o,e]*x[r0+n,e] + B[o,e]*x[r0+1+n,e]
                    nc.tensor.matmul(
                        pm[:],
                        lhsT=wa_sb,
                        rhs=sample_windows(b, r0, CW),
                        start=True,
                        stop=False,
                    )
                    nc.tensor.matmul(
                        pm[:],
                        lhsT=wb_sb,
                        rhs=sample_windows(b, r0 + 1, CW),
                        start=False,
                        stop=True,
                    )
                    # PSUM -> SBUF downcast + bias, alternating ACT/DVE.
                    # k==7 writes only its last 511 cols (out cols 3585..4095)
                    # so it never overlaps k==6's 3073..3584.
                    if k < N_CHUNKS - 1:
                        src = pm[:]
                        if not last:
                            dst = obig[:, 1 + r0 : 1 + r0 + CW]
                        else:
                            g, off = divmod(r0, 1024)
                            dst = ogs[g][:, off : off + CW]
                    else:
                        src = pm[:, 1:CW]
                        if not last:
                            dst = obig[:, 3585:L]
                        else:
                            dst = ogs[3][:, 512:1023]
                    if k % 2 == 0:
                        nc.scalar.add(dst, src, cb_sb[:])
                    else:
                        nc.vector.tensor_scalar_add(dst, src, cb_sb[:])

                if not last:
                    # col 0 is garbage; host overwrites output row 0 anyway.
                    # full-width store = aligned 8KB/partition lines.
                    nc.gpsimd.dma_start(out=out[b], in_=obig[:])
                else:
                    nc.gpsimd.dma_start(out=out[b, :, 1:1025], in_=ogs[0][:])
                    nc.gpsimd.dma_start(out=out[b, :, 1025:2049], in_=ogs[1][:])
                    nc.gpsimd.dma_start(out=out[b, :, 2049:3073], in_=ogs[2][:])
                    nc.gpsimd.dma_start(out=out[b, :, 3073:L], in_=ogs[3][:])

    nc.compile()
    return nc


# test.py toggles these to capture an NTFF/perfetto profile of the run; the
# grading harness never touches them (TRACE defaults False).
TRACE = False
TRACE_CORES = None  # e.g. [0] or list(range(N_CORES))
TRACE_TMPDIR = None
LAST_RESULT = None

_NC_CACHE = {}


def _get_program():
    if "nc" not in _NC_CACHE:
        _NC_CACHE["nc"] = _build_program()
    return _NC_CACHE["nc"]


def kernel(loc, W_src, W_dst, attn_l, attn_r, W_res, bias):
    loc = np.asarray(loc, dtype=np.float32)
    H = 8
    A = np.asarray(W_src, np.float32).reshape(H, F, F).mean(axis=0)
    Bm = np.asarray(W_res, np.float32).reshape(H, F, F).mean(axis=0)
    c = np.asarray(bias, np.float32).reshape(H, F).mean(axis=0)

    # feature-major bf16 inputs for the device (features on SBUF partitions)
    xt_full = np.ascontiguousarray(
        loc.transpose(0, 2, 1).astype(NP_BF16)
    )  # (B, F, L)
    # wab[e, 0:128] = A[:, e] (i.e. A^T), wab[e, 128:256] = B^T
    wab = np.ascontiguousarray(
        np.concatenate([A.T, Bm.T], axis=1).astype(NP_BF16)
    )
    cbv = np.ascontiguousarray(c.reshape(F, 1))

    in_maps = [
        {
            "xt": np.ascontiguousarray(xt_full[i * B_SH : (i + 1) * B_SH]),
            "wab": wab,
            "cb": cbv,
        }
        for i in range(N_CORES)
    ]

    nc = _get_program()
    kw = {}
    if TRACE:
        kw = dict(
            trace=True,
            trace_cores=TRACE_CORES if TRACE_CORES is not None else [0],
            tmpdir=TRACE_TMPDIR,
        )
    res = run_bass_kernel_spmd(nc, in_maps, list(range(N_CORES)), **kw)
    if TRACE:
        global LAST_RESULT
        LAST_RESULT = res

    out = np.empty((B_FULL, L, F), dtype=np.float32)
    for i in range(N_CORES):
        out[i * B_SH : (i + 1) * B_SH] = (
            res.results[i]["out"].astype(np.float32).transpose(0, 2, 1)
        )
    out[:, 0, :] = loc[:, 0, :]  # origin row passthrough
    return out


# revision 14
# speedup vs baseline: 1.6535x; 1.0147x over previous
"""Trainium2 Bass kernel for nn_GAT_14946486190732.

Math: the reference builds a chain graph where edge i connects src node i to
dst node i (u = v = arange(E)), so every dst segment in the edge softmax has
exactly one edge: segment_max == the score itself, exp(0) == 1, denom == 1,
alpha == 1 exactly. The whole attention branch is a no-op, and

    out[b, 0,  :] = loc[b, 0, :]
    out[b, i,  :] = loc[b, i-1, :] @ A^T + loc[b, i, :] @ B^T + c   (i >= 1)

with A = mean_h W_src.reshape(H,F,F), B = mean_h W_res.reshape(H,F,F),
c = mean_h bias.reshape(H,F)  (head-mean folded into the weights).

Device strategy (8 cores, data-parallel over batch, 4 samples/core):
  - HBM-bandwidth-bound, so the streamed tensors travel as bfloat16 (host
    pre-transposes loc to (B, F, L) + casts; device returns feature-major
    bf16, host un-transposes/upcasts). Rel err ~2.6e-3 vs the 2e-2 gate.
  - loads issue from the idle sync engine (HWDGE queue) so they are never
    stuck behind store packets; stores issue from gpsimd (SWDGE queue).
  - sample 0's load is split in two so the first matmul starts as soon as
    ~0.5 MB lands; samples 1-3 load as single 1 MB DMAs (8 KB/partition
    lines, no runt packets). Stores are whole-sample for samples 0-2 and
    1024-col groups for the last sample to shorten the drain tail.
  - per 512-col chunk: two PSUM-accumulated bf16 matmuls (full PE rate,
    weights stationary); bias-add fused into the PSUM->SBUF downcast copy,
    alternating ACT/DVE; ACT's activation table is pre-warmed during the
    first load so the table load is off the critical path.
"""

import numpy as np
import ml_dtypes

from concourse import bass, bacc, tile, mybir
from concourse.bass_utils import run_bass_kernel_spmd

F32 = mybir.dt.float32
BF16 = mybir.dt.bfloat16
NP_BF16 = ml_dtypes.bfloat16

N_CORES = 8
B_FULL, L, F = 32, 4096, 128
B_SH = B_FULL // N_CORES  # samples per core
N_CHUNKS = 8  # 512-col matmul chunks per sample
CW = 512  # matmul chunk width (one PSUM bank: 512 * 4B = 2KB/partition)
# sample-0 load pieces: [0,1025) [1024,2049) [2048,3073) [3072,4096) — each
# new piece starts one col before the previous ends so every 513-col matmul
# window lives inside a single piece
S0_STARTS = (0, 1024, 2048, 3072)
S0_WIDTHS = (1025, 1025, 1025, 1024)
N_WARM = 6  # junk matmuls during the load window to start the PE DVFS ramp


def _build_program():
    nc = bacc.Bacc(
        "TRN2",
        target_bir_lowering=False,
        num_devices=N_CORES,
        num_swdge_queues=4,
    )

    xt = nc.declare_dram_parameter("xt", [B_SH, F, L], BF16, isOutput=False)
    # wab[:, 0:128] = A^T, wab[:, 128:256] = B^T (single load DMA)
    wab = nc.declare_dram_parameter("wab", [F, 2 * F], BF16, isOutput=False)
    cb = nc.declare_dram_parameter("cb", [F, 1], F32, isOutput=False)
    out = nc.declare_dram_parameter("out", [B_SH, F, L], BF16, isOutput=True)

    with tile.TileContext(nc) as tc:
        with (
            tc.tile_pool(name="consts", bufs=1) as consts,
            tc.tile_pool(name="xtp", bufs=3) as xtp,  # whole-sample tiles
            tc.tile_pool(name="xsp", bufs=4) as xsp,  # sample-0 split tiles
            tc.tile_pool(name="obig", bufs=3) as obigp,
            tc.tile_pool(name="ostg", bufs=4) as ostgp,  # last-sample groups
            tc.tile_pool(name="pmm", bufs=6, space="PSUM") as pmmp,
            tc.tile_pool(name="pwarm", bufs=1, space="PSUM") as pwarmp,
        ):
            wab_sb = consts.tile([F, 2 * F], BF16)
            cb_sb = consts.tile([F, 1], F32)
            warm = consts.tile([F, 1], F32)
            junk = consts.tile([F, 256], BF16)

            wa_sb = wab_sb[:, 0:F]
            wb_sb = wab_sb[:, F : 2 * F]

            # ---- loads (sync engine / HWDGE): issue everything up front ----
            # order: weights, then sample 0 in 4 pieces (first matmul starts
            # after ~256KB), bias, then samples 1-3 whole
            nc.sync.dma_start(out=wab_sb[:], in_=wab[:])
            x0p = []
            for j, (s0, w0) in enumerate(zip(S0_STARTS, S0_WIDTHS)):
                if j == 1:
                    nc.sync.dma_start(out=cb_sb[:], in_=cb[:])
                t = xsp.tile([F, w0], BF16, name=f"x0p{j}")
                nc.sync.dma_start(out=t[:], in_=xt[0, :, s0 : s0 + w0])
                x0p.append(t)
            xts = []
            for b in range(1, B_SH):
                t = xtp.tile([F, L], BF16, name=f"xt{b}")
                nc.sync.dma_start(out=t[:], in_=xt[b])
                xts.append(t)

            # PE DVFS pre-warm: junk matmuls with no load dependency keep the
            # PE busy during the first load so the clock ramp starts early
            nc.gpsimd.memset(junk[:], 0)
            pw = pwarmp.tile([F, 256], F32)
            for i in range(N_WARM):
                nc.tensor.matmul(
                    pw[:], lhsT=junk[:, 0:F], rhs=junk[:], start=True, stop=True
                )
            # pull ACT's activation-table load off the critical path: runs
            # while sample 0 is still streaming in
            nc.scalar.add(warm[:], cb_sb[:], cb_sb[:])

            def sample_windows(b, r0, w):
                """SBUF view of xt[b, :, r0:r0+w] given the load tiling."""
                if b == 0:
                    j = r0 // 1024
                    if j < 3 and r0 + w > S0_STARTS[j] + S0_WIDTHS[j]:
                        j += 1  # window crosses into the next piece's overlap
                    off = r0 - S0_STARTS[j]
                    assert 0 <= off and off + w <= S0_WIDTHS[j], (r0, w, j)
                    return x0p[j][:, off : off + w]
                return xts[b - 1][:, r0 : r0 + w]

            # ---- compute + stores ----
            for b in range(B_SH):
                last = b == B_SH - 1
                if not last:
                    obig = obigp.tile([F, L], BF16)
                    ogs = None
                else:
                    # 1024-col store groups: dest cols 1/1025/2049/3073
                    ogs = [
                        ostgp.tile([F, 1024 if g < 3 else 1023], BF16, name=f"og{g}")
                        for g in range(4)
                    ]

                for k in range(N_CHUNKS):
                    # last chunk starts one col early so all chunks are 512
                    # wide; its first output col is dropped at the copy.
                    r0 = CW * k if k < N_CHUNKS - 1 else L - 1 - CW
                    pm = pmmp.tile([F, CW], F32)
                    # pm[o, n] = sum_e A[o,e]*x[r0+n,e] + B[o,e]*x[r0+1+n,e]
                    nc.tensor.matmul(
                        pm[:],
                        lhsT=wa_sb,
                        rhs=sample_windows(b, r0, CW),
                        start=True,
                        stop=False,
                    )
                    nc.tensor.matmul(
                        pm[:],
                        lhsT=wb_sb,
                        rhs=sample_windows(b, r0 + 1, CW),
                        start=False,
                        stop=True,
                    )
                    # PSUM -> SBUF downcast + bias, alternating ACT/DVE.
                    # k==7 writes only its last 511 cols (out cols 3585..4095)
                    # so it never overlaps k==6's 3073..3584.
                    if k < N_CHUNKS - 1:
                        src = pm[:]
                        if not last:
                            dst = obig[:, 1 + r0 : 1 + r0 + CW]
                        else:
                            g, off = divmod(r0, 1024)
                            dst = ogs[g][:, off : off + CW]
                    else:
                        src = pm[:, 1:CW]
                        if not last:
                            dst = obig[:, 3585:L]
                        else:
                            dst = ogs[3][:, 512:1023]
                    if k % 2 == 0:
                        nc.scalar.add(dst, src, cb_sb[:])
                    else:
                        nc.vector.tensor_scalar_add(dst, src, cb_sb[:])

                if not last:
                    # col 0 is garbage; host overwrites output row 0 anyway.
                    # full-width store = aligned 8KB/partition lines.
                    nc.gpsimd.dma_start(out=out[b], in_=obig[:])
                else:
                    nc.gpsimd.dma_start(out=out[b, :, 1:1025], in_=ogs[0][:])
                    nc.gpsimd.dma_start(out=out[b, :, 1025:2049], in_=ogs[1][:])
                    nc.gpsimd.dma_start(out=out[b, :, 2049:3073], in_=ogs[2][:])
                    nc.gpsimd.dma_start(out=out[b, :, 3073:L], in_=ogs[3][:])

    nc.compile()
    return nc


# test.py toggles these to capture an NTFF/perfetto profile of the run; the
# grading harness never touches them (TRACE defaults False).
TRACE = False
TRACE_CORES = None  # e.g. [0] or list(range(N_CORES))
TRACE_TMPDIR = None
LAST_RESULT = None

_NC_CACHE = {}


def _get_program():
    if "nc" not in _NC_CACHE:
        _NC_CACHE["nc"] = _build_program()
    return _NC_CACHE["nc"]


def kernel(loc, W_src, W_dst, attn_l, attn_r, W_res, bias):
    loc = np.asarray(loc, dtype=np.float32)
    H = 8
    A = np.asarray(W_src, np.float32).reshape(H, F, F).mean(axis=0)
    Bm = np.asarray(W_res, np.float32).reshape(H, F, F).mean(axis=0)
    c = np.asarray(bias, np.float32).reshape(H, F).mean(axis=0)

    # feature-major bf16 inputs for the device (features on SBUF partitions)
    xt_full = np.ascontiguousarray(
        loc.transpose(0, 2, 1).astype(NP_BF16)
    )  # (B, F, L)
    # wab[e, 0:128] = A[:, e] (i.e. A^T), wab[e, 128:256] = B^T
    wab = np.ascontiguousarray(
        np.concatenate([A.T, Bm.T], axis=1).astype(NP_BF16)
    )
    cbv = np.ascontiguousarray(c.reshape(F, 1))

    in_maps = [
        {
            "xt": np.ascontiguousarray(xt_full[i * B_SH : (i + 1) * B_SH]),
            "wab": wab,
            "cb": cbv,
        }
        for i in range(N_CORES)
    ]

    nc = _get_program()
    kw = {}
    if TRACE:
        kw = dict(
            trace=True,
            trace_cores=TRACE_CORES if TRACE_CORES is not None else [0],
            tmpdir=TRACE_TMPDIR,
        )
    res = run_bass_kernel_spmd(nc, in_maps, list(range(N_CORES)), **kw)
    if TRACE:
        global LAST_RESULT
        LAST_RESULT = res

    out = np.empty((B_FULL, L, F), dtype=np.float32)
    for i in range(N_CORES):
        out[i * B_SH : (i + 1) * B_SH] = (
            res.results[i]["out"].astype(np.float32).transpose(0, 2, 1)
        )
    out[:, 0, :] = loc[:, 0, :]  # origin row passthrough
    return out
